# revision 18
# baseline (speedup 1.0000x reference)
"""Bass/Tile TRN2 kernel for nn_Network_21131239096982 (gnn_message_passing).

Sharding: 8 cores = 4 samples x 2 y-halves (full x per core). Pair order
(y outer, x inner). Per-layer ReduceScatter(cc_dim=Free) over the pair
sums the y-half partial preactivations and hands each core exactly its
own y-half of the next layer's features (rank0 -> cols 0:64, rank1 ->
cols 64:128), so the SPMD program needs no per-core offsets. Layer 3
does an 8-way AllGather of the [32, 128] partials; each core sums the
pair blocks and runs the batchnorm MLP head redundantly.

Key restructure vs the reference: the radial MLP (cosine basis -> ssp
-> ssp -> w3) is 128 univariate functions of r, host-tabulated on a
128-point r-uniform grid and evaluated on device as ONE matmul per pair
chunk:  s2[h, pair] = Ftab_l[g, h]^T @ O[g, pair],  where O holds
linear-interpolation hat weights in u = r^2 (asymmetric triangular
hats -> no sqrt needed). O is built once from a rank-1 PE broadcast
psum = sl_g * u plus three DVE ops per chunk. The gate
softplus(5t)/5 = relu(t) + poly9(min(|t|,2)) runs on DVE (+ Abs/Relu,
present in every ACT table set), so the only ACT table load in the
whole kernel is ln/exp for the two batchnorm inverse-stddevs.
"""

import math

import numpy as np

B, N, EMB, MUL = 4, 128, 32, 32
NB, MAXR = 10, 10.0
HID, BETA = 128, 5.0
MID, OUT = 256, 128
NL = 4
Y0 = 1.0 / (2.0 * math.sqrt(math.pi))
YH = N // 2          # 64 local y's per core
NP = N * YH          # 8192 pairs per core, order (y outer, x inner)
NCORES = 8
GRID_N = 128
RMAX = 7.5
SQN = 1.0 / math.sqrt(N)

# softplus(5t)/5 - relu(t) = ln(1+exp(-5|t|))/5, chebyshev fit on [0, 2]
GATE_PC = [0.13863592819866152, -0.4999284878393997, 0.6156649023363564,
           0.12337920499527943, -1.3216523110767724, 1.8311453040108088,
           -1.3266478452560657, 0.5557922376483523, -0.1274729154222193,
           0.012418893315223408]

_cached = None


def _build():
    import jax

    jax.devices()  # axon boot
    from concourse import bacc, tile, mybir

    F32 = mybir.dt.float32
    BF16 = mybir.dt.bfloat16
    AF = mybir.ActivationFunctionType
    ALU = mybir.AluOpType

    nc = bacc.Bacc("TRN2", debug=False, num_devices=NCORES)

    def din(name, shape, dt=F32):
        return nc.dram_tensor(name, shape, dt, kind="ExternalInput").ap()

    geoY_d = din("geoY", [5, YH])
    geoX_d = din("geoX", [5, N])
    slrow_d = din("slrow", [1, GRID_N], BF16)
    coef_d = din("coef", [GRID_N, 3])          # A-add, B-mult, B-add
    ftab_d = din("ftab", [GRID_N, NL * HID], BF16)
    wg_d = din("wg", [MUL, NL * MUL * HID], BF16)
    fm0_d = din("fm0", [MUL, YH], BF16)        # my y-half of fm layer 0
    msqn_d = din("msqn", [YH, MUL])            # mask_half/sqrt(N) x ones32
    ident64_d = din("ident64", [YH, YH])
    mhead_d = din("mhead", [MUL, B * N])
    w1c_d = din("w1c", [EMB, MID], BF16)
    b1c_d = din("b1c", [128, 2])
    w2c_d = din("w2c", [128, MID], BF16)
    b2c_d = din("b2c", [128, 1])
    wbar1_d = din("wbar1", [EMB, 1], BF16)
    wbar2_d = din("wbar2", [128, 2], BF16)
    bbar_d = din("bbar", [1, 2])
    gb1_d = din("gb1", [1, 2 * N])             # g1r | be1r
    gb2_d = din("gb2", [1, 2 * N])             # g2r | be2r
    ones128_d = din("ones128", [128, 1], BF16)
    ones1_d = din("ones1", [1, 128], BF16)
    maskB_d = din("maskB", [128, B * N], BF16)
    mvB_d = din("mvB", [128, B])
    epsv_d = din("epsv", [1, 1])
    out_d = nc.dram_tensor("out", [B, OUT], F32, kind="ExternalOutput").ap()

    UMAX = (RMAX ** 2) * (1.0 - 1e-4)
    CH = 1024            # pair columns per psum tile (2 matmuls of 512)
    NCH = NP // CH       # 8

    with tile.TileContext(nc) as tc:
        with (
            tc.tile_pool(name="const", bufs=1) as cp,
            tc.tile_pool(name="s2p", bufs=3) as s2p,
            tc.tile_pool(name="gbp", bufs=2) as gbp,
            tc.tile_pool(name="fmp", bufs=2) as fmp,
            tc.tile_pool(name="wk", bufs=2) as wk,
            tc.tile_pool(name="hd", bufs=2) as hd,
            tc.tile_pool(name="ps_big", bufs=2, space="PSUM") as pA,
            tc.tile_pool(name="ps_g", bufs=2, space="PSUM") as pG,
            tc.tile_pool(name="ps_fc", bufs=2, space="PSUM") as pF,
            tc.tile_pool(name="dram", bufs=1, space="DRAM") as dp,
        ):
            def cload(ap, shape, dt=F32):
                t = cp.tile(shape, dt, name=ap.tensor.name + "_sb")
                nc.sync.dma_start(t[:], ap[:])
                return t

            geoY = cload(geoY_d, [5, YH])
            geoX = cload(geoX_d, [5, N])
            slrow = cload(slrow_d, [1, GRID_N], BF16)
            coef = cload(coef_d, [GRID_N, 3])
            fm0 = cload(fm0_d, [MUL, YH], BF16)
            msqn = cload(msqn_d, [YH, MUL])
            ident64 = cload(ident64_d, [YH, YH])
            ftab = cload(ftab_d, [GRID_N, NL * HID], BF16)
            mhead = cload(mhead_d, [MUL, B * N])
            w1c = cload(w1c_d, [EMB, MID], BF16)
            b1c = cload(b1c_d, [128, 2])
            w2c = cload(w2c_d, [128, MID], BF16)
            b2c = cload(b2c_d, [128, 1])
            wbar1 = cload(wbar1_d, [EMB, 1], BF16)
            wbar2 = cload(wbar2_d, [128, 2], BF16)
            bbar = cload(bbar_d, [1, 2])
            gb1 = cload(gb1_d, [1, 2 * N])
            gb2 = cload(gb2_d, [1, 2 * N])
            ones128 = cload(ones128_d, [128, 1], BF16)
            ones1 = cload(ones1_d, [1, 128], BF16)
            maskB = cload(maskB_d, [128, B * N], BF16)
            mvB = cload(mvB_d, [128, B])
            epsv = cload(epsv_d, [1, 1])

            # ---- act-table warmup (ln/exp set used by the BN head) ----
            tblw = wk.tile([1, 1], F32, name="tblw", tag="tblw", bufs=1)
            nc.scalar.activation(tblw[:], epsv[:], AF.Ln, bias=1.0)

            # ---- u = r^2 [y, x] clamped bf16; flatten via DRAM bounce ----
            r2ps = pG.tile([YH, N], F32, name="r2ps", tag="g")
            nc.tensor.matmul(r2ps[:], geoY[:], geoX[:], start=True, stop=True)
            u2d = wk.tile([YH, N], BF16, name="u2d", tag="u2d")
            nc.vector.tensor_scalar(
                u2d[:], r2ps[:], 0.0, UMAX, op0=ALU.max, op1=ALU.min)
            ubounce = dp.tile([YH, N], BF16, name="ubounce")
            nc.sync.dma_start(ubounce[:], u2d[:])
            urow = cp.tile([1, NP], BF16, name="urow")
            nc.sync.dma_start(
                urow[:], ubounce.opt().rearrange("p x -> () (p x)"))

            # ---- O[g, pair]: linear-interp hats in u ----
            # psum = sl_g*u ; A = psum + coef0 ; B = psum*coef1 + coef2
            # O = relu(min(A, B))
            obuf = cp.tile([GRID_N, NP], BF16, name="obuf")
            for c in range(NCH):
                ups = pA.tile([GRID_N, CH], F32, name="ups", tag="big")
                for h in range(2):
                    nc.tensor.matmul(
                        ups[:, h * 512:(h + 1) * 512], slrow[:],
                        urow[:, c * CH + h * 512:c * CH + (h + 1) * 512],
                        start=True, stop=True)
                osl = obuf[:, c * CH:(c + 1) * CH]
                bt = wk.tile([GRID_N, CH], F32, name="btile", tag="btile")
                nc.vector.tensor_scalar(
                    bt[:], ups[:], coef[:, 1:2], coef[:, 2:3],
                    op0=ALU.mult, op1=ALU.add)
                nc.vector.scalar_tensor_tensor(
                    osl, ups[:], coef[:, 0:1], bt[:],
                    op0=ALU.add, op1=ALU.min)
                nc.vector.tensor_scalar(osl, osl, 0.0, None, op0=ALU.max)

            # ---- gate helper ----
            def gate_chain(src_ap, pdim, width, mask_ap, name, res_dt=BF16):
                tt = wk.tile([pdim, width], F32, name=f"tt{name}", tag="gt", bufs=6)
                nc.scalar.activation(tt[:], src_ap, AF.Abs)
                nc.vector.tensor_scalar(tt[:], tt[:], 2.0, None, op0=ALU.min)
                # recurrence q=(q+c)*u gives a9*u^9+(c1)u^8+...+(c8)u,
                # so feed c_j = a_{9-j}; a0 folds into the final mask STT.
                pv = wk.tile([pdim, width], F32, name=f"pv{name}", tag="gt", bufs=6)
                nc.vector.tensor_scalar(
                    pv[:], tt[:], GATE_PC[9], None, op0=ALU.mult)
                for k in range(8, 0, -1):
                    nc.vector.scalar_tensor_tensor(
                        pv[:], pv[:], GATE_PC[k], tt[:],
                        op0=ALU.add, op1=ALU.mult)
                rl = wk.tile([pdim, width], F32, name=f"rl{name}", tag="gt", bufs=6)
                nc.scalar.activation(rl[:], src_ap, AF.Relu)
                nc.vector.tensor_tensor(pv[:], pv[:], rl[:], op=ALU.add)
                res = fmp.tile([pdim, width], res_dt, name=f"fm{name}", tag="fm")
                nc.vector.scalar_tensor_tensor(
                    res[:], pv[:], GATE_PC[0], mask_ap,
                    op0=ALU.add, op1=ALU.mult)
                return res

            # ---- conv layers ----
            fm = [fm0] + [None] * NL
            part3 = None

            def load_wg(l):
                t = gbp.tile([MUL, MUL * HID], BF16, name=f"wg{l}", tag="wg", bufs=4)
                nc.sync.dma_start(
                    t[:], wg_d[:, l * MUL * HID:(l + 1) * MUL * HID])
                return t

            def radial(l):
                s2 = s2p.tile([HID, NP], BF16, name=f"s2_{l}", tag="s2")
                for c in range(NCH):
                    rps = pA.tile([HID, CH], F32, name="rps", tag="big")
                    for h in range(2):
                        nc.tensor.matmul(
                            rps[:, h * 512:(h + 1) * 512],
                            ftab[:, l * HID:(l + 1) * HID],
                            obuf[:, c * CH + h * 512:c * CH + (h + 1) * 512],
                            start=True, stop=True)
                    dst = s2[:, c * CH:(c + 1) * CH]
                    if c % 2 == 0:
                        nc.scalar.activation(dst, rps[:], AF.Copy)
                    else:
                        nc.vector.tensor_copy(dst, rps[:])
                return s2

            wgl = [load_wg(l) for l in range(NL)]
            s2s = [None] * NL
            s2s[0] = radial(0)
            s2s[1] = radial(1)
            for l in range(NL):
                s2 = s2s[l]
                wg = wgl[l]
                # G-stage: gbuf[h, (i, y)] in blocks of 4 i's
                gbuf = gbp.tile([HID, MUL * YH], BF16, name=f"gb{l}", tag="gb")
                for q in range(MUL // 4):
                    gps = pG.tile([HID, 4 * YH], F32, name="gps", tag="g")
                    for k in range(4):
                        i = q * 4 + k
                        nc.tensor.matmul(
                            gps[:, k * YH:(k + 1) * YH],
                            wg[:, i * HID:(i + 1) * HID],
                            fm[l][:], start=True, stop=True)
                    nc.vector.tensor_copy(
                        gbuf[:, q * 4 * YH:(q + 1) * 4 * YH], gps[:])

                # final contraction over my y-half
                gview = gbuf[:].rearrange("p (i y) -> p y i", y=YH)
                if l < NL - 1:
                    # transposed partial [x, i]: flat RS halves == y-halves
                    pf = pF.tile([N, MUL], F32, name=f"pf{l}", tag="fc")
                    for y in range(YH):
                        nc.tensor.matmul(
                            pf[:], s2[:, y * N:(y + 1) * N], gview[:, y, :],
                            start=(y == 0), stop=(y == YH - 1))
                    part = wk.tile([N, MUL], F32, name=f"part{l}", tag="part")
                    nc.vector.tensor_copy(part[:], pf[:])
                    ari = dp.tile([N, MUL], F32, name=f"ari{l}")
                    nc.sync.dma_start(ari[:], part[:])
                    aro = dp.tile([YH, MUL], F32, name=f"aro{l}")
                    nc.gpsimd.collective_compute(
                        "ReduceScatter", ALU.add,
                        replica_groups=[[0, 1], [2, 3], [4, 5], [6, 7]],
                        ins=[ari.opt()], outs=[aro.opt()], cc_dim="Free")
                    pre = wk.tile([YH, MUL], F32, name=f"pre{l}", tag="pre")
                    nc.sync.dma_start(pre[:], aro.opt())
                    gfm = gate_chain(pre[:], YH, MUL, msqn[:], f"{l}", res_dt=F32)
                    # transpose [y, i] -> fm [i, y] for the next G-stage
                    ftp = pG.tile([MUL, YH], F32, name=f"ftp{l}", tag="g")
                    nc.tensor.transpose(ftp[:], gfm[:], ident64[:])
                    fmn = fmp.tile([MUL, YH], BF16, name=f"fmn{l}", tag="fm")
                    nc.vector.tensor_copy(fmn[:], ftp[:])
                    fm[l + 1] = fmn
                    if l + 2 < NL:
                        s2s[l + 2] = radial(l + 2)
                else:
                    pf = pF.tile([MUL, N], F32, name=f"pf{l}", tag="fc")
                    for y in range(YH):
                        nc.tensor.matmul(
                            pf[:], gview[:, y, :], s2[:, y * N:(y + 1) * N],
                            start=(y == 0), stop=(y == YH - 1))
                    part = wk.tile([MUL, N], F32, name=f"part{l}", tag="part")
                    nc.vector.tensor_copy(part[:], pf[:])
                    part3 = part

            # ---- layer-3 combine: 8-way AllGather, sum pair halves ----
            wrm2 = dp.tile([1, 64], F32, name="wrm2")
            wrm2o = dp.tile([NCORES, 64], F32, name="wrm2o")
            nc.sync.dma_start(wrm2[:], coef_d[:, 1:2].rearrange("p o -> o p")[:, 0:64])
            nc.gpsimd.collective_compute(
                "AllGather", ALU.bypass,
                replica_groups=[list(range(NCORES))],
                ins=[wrm2.opt()], outs=[wrm2o.opt()])
            ag3i = dp.tile([MUL, N], F32, name="ag3i")
            nc.sync.dma_start(ag3i[:], part3[:])
            ag3o = dp.tile([NCORES * MUL, N], F32, name="ag3o")
            nc.gpsimd.collective_compute(
                "AllGather", ALU.bypass,
                replica_groups=[list(range(NCORES))],
                ins=[ag3i.opt()], outs=[ag3o.opt()])
            agv = ag3o.opt().rearrange("(b h i) x -> h i b x", h=2, i=MUL)
            t3a = hd.tile([MUL, B * N], F32, name="t3a", tag="t3")
            t3b = hd.tile([MUL, B * N], F32, name="t3b", tag="t3")
            nc.sync.dma_start(
                t3a[:].rearrange("i (b x) -> i b x", b=B), agv[0])
            nc.sync.dma_start(
                t3b[:].rearrange("i (b x) -> i b x", b=B), agv[1])
            nc.vector.tensor_tensor(t3a[:], t3a[:], t3b[:], op=ALU.add)
            fT = gate_chain(t3a[:], MUL, B * N, mhead[:], "hd")

            # ---- head: 2x (linear + BN + lrelu), masked mean pool ----
            BN_ = B * N

            def bn_block(rhs_tiles, wts, wbars, bbar_ap, bias, gbrow, cnt,
                         nchunk):
                """rhs_tiles: bf16 [K, BN_] chunks; wts[m][kk]: lhsT APs;
                wbars[kk]: [K, 1] col-sum lhsT APs."""
                mups = pG.tile([1, BN_], F32, name="mups", tag="g")
                for kk, rt in enumerate(rhs_tiles):
                    nc.tensor.matmul(
                        mups[:], wbars[kk], rt[:],
                        start=(kk == 0), stop=(kk == len(rhs_tiles) - 1))
                mu = hd.tile([1, N], F32, name="mu", tag="r128", bufs=10)
                nc.vector.reduce_sum(
                    mu[:], mups[:].rearrange("p (b x) -> p x b", b=B),
                    axis=mybir.AxisListType.X)
                nc.vector.tensor_scalar(
                    mu[:], mu[:], 1.0 / cnt, bbar_ap, op0=ALU.mult, op1=ALU.add)
                asb, sqs = [], []
                for m in range(nchunk):
                    aps = pA.tile([128, BN_], F32, name=f"aps{m}", tag="big")
                    for kk, rt in enumerate(rhs_tiles):
                        nc.tensor.matmul(
                            aps[:], wts[m][kk], rt[:],
                            start=(kk == 0),
                            stop=(kk == len(rhs_tiles) - 1))
                    av = hd.tile([128, BN_], F32, name=f"av{m}", tag="av", bufs=3)
                    nc.vector.tensor_scalar(
                        av[:], aps[:], bias[:, m:m + 1], None, op0=ALU.add)
                    sq = hd.tile([128, BN_], BF16, name=f"sq{m}", tag="sq", bufs=3)
                    nc.scalar.activation(
                        sq[:], aps[:], AF.Square, bias=bias[:, m:m + 1])
                    asb.append(av)
                    sqs.append(sq)
                qps = pG.tile([1, BN_], F32, name="qps", tag="g")
                for m in range(nchunk):
                    nc.tensor.matmul(qps[:], ones128[:], sqs[m][:],
                                     start=(m == 0), stop=(m == nchunk - 1))
                var = hd.tile([1, N], F32, name="var", tag="r128", bufs=10)
                nc.vector.reduce_sum(
                    var[:], qps[:].rearrange("p (b x) -> p x b", b=B),
                    axis=mybir.AxisListType.X)
                nc.vector.tensor_scalar_mul(var[:], var[:], 1.0 / cnt)
                musq = hd.tile([1, N], F32, name="musq", tag="r128", bufs=10)
                nc.vector.tensor_tensor(musq[:], mu[:], mu[:], op=ALU.mult)
                nc.vector.tensor_tensor(
                    var[:], var[:], musq[:], op=ALU.subtract)
                inv = hd.tile([1, N], F32, name="inv", tag="r128", bufs=10)
                nc.scalar.activation(inv[:], var[:], AF.Ln, bias=epsv[:, 0:1])
                nc.scalar.activation(inv[:], inv[:], AF.Exp, scale=-0.5)
                svec = hd.tile([1, N], F32, name="svec", tag="r128", bufs=10)
                nc.vector.tensor_tensor(
                    svec[:], gbrow[:, 0:N], inv[:], op=ALU.mult)
                tvec = hd.tile([1, N], F32, name="tvec", tag="r128", bufs=10)
                nc.vector.tensor_tensor(tvec[:], mu[:], svec[:], op=ALU.mult)
                nc.vector.scalar_tensor_tensor(
                    tvec[:], tvec[:], -1.0, gbrow[:, N:2 * N],
                    op0=ALU.mult, op1=ALU.add)
                sw = hd.tile([1, BN_], BF16, name="sw", tag="row", bufs=6)
                tw = hd.tile([1, BN_], BF16, name="tw", tag="row", bufs=6)
                for b in range(B):
                    nc.vector.tensor_copy(sw[:, b * N:(b + 1) * N], svec[:])
                    nc.vector.tensor_copy(tw[:, b * N:(b + 1) * N], tvec[:])
                sB = pG.tile([128, BN_], F32, name="sB", tag="g")
                tB = pG.tile([128, BN_], F32, name="tB", tag="g")
                nc.tensor.matmul(sB[:], ones1[:], sw[:], start=True, stop=True)
                nc.tensor.matmul(tB[:], ones1[:], tw[:], start=True, stop=True)
                outs = []
                for m in range(nchunk):
                    nc.vector.tensor_tensor(
                        asb[m][:], asb[m][:], sB[:], op=ALU.mult)
                    nc.vector.tensor_tensor(
                        asb[m][:], asb[m][:], tB[:], op=ALU.add)
                    ho = hd.tile([128, BN_], BF16, name=f"ho{m}", tag="ho", bufs=4)
                    nc.scalar.activation(ho[:], asb[m][:], AF.Prelu, alpha=0.2)
                    outs.append(ho)
                return outs

            w1chunks = [[w1c[:, m * 128:(m + 1) * 128]] for m in range(2)]
            h1 = bn_block([fT], w1chunks, [wbar1[:]], bbar[:, 0:1],
                          b1c, gb1, float(B * MID), 2)
            w2chunks = [[w2c[:, k * 128:(k + 1) * 128] for k in range(2)]]
            h2 = bn_block(h1, w2chunks,
                          [wbar2[:, 0:1], wbar2[:, 1:2]], bbar[:, 1:2],
                          b2c, gb2, float(B * 128), 1)

            # masked mean pool -> out[b, o]
            h2m = hd.tile([128, BN_], BF16, name="h2m", tag="ho", bufs=4)
            nc.vector.tensor_tensor(h2m[:], h2[0][:], maskB[:], op=ALU.mult)
            pool = hd.tile([128, B], F32, name="pool", tag="pool")
            nc.vector.reduce_sum(
                pool[:], h2m[:].rearrange("p (b x) -> p b x", b=B),
                axis=mybir.AxisListType.X)
            nc.vector.tensor_tensor(pool[:], pool[:], mvB[:], op=ALU.mult)
            nc.sync.dma_start(out_d[:].rearrange("b o -> o b"), pool[:])

    nc.compile()
    return nc


def _ssp_chain(r, rw1, rw2, rw3_unused):
    grid = np.linspace(0.0, MAXR, NB)
    step = grid[1] - grid[0]
    x = (r[..., None] - grid) / step
    basis = np.where(np.abs(x) < 1.0, np.cos(0.5 * math.pi * x) ** 2, 0.0)

    def ssp(v):
        return (np.logaddexp(0, BETA * v) - math.log(2.0)) / BETA

    h = ssp(basis @ rw1 / math.sqrt(NB))
    h = ssp(h @ rw2 / math.sqrt(HID))
    return h


def _host_prep(inputs):
    import ml_dtypes
    BF = ml_dtypes.bfloat16
    f32 = np.float32

    f = {k: np.asarray(v) for k, v in inputs.items()}
    geometry = f["geometry"].astype(np.float64)
    features = f["features"].astype(np.int64)
    mask = f["mask"].astype(np.float64)
    emb = f["emb"].astype(np.float64)
    rw1, rw2, rw3 = (f[k].astype(np.float64) for k in ("rw1", "rw2", "rw3"))
    W1, b1 = f["W1"].astype(np.float64), f["b1"].astype(np.float64)
    W2, b2 = f["W2"].astype(np.float64), f["b2"].astype(np.float64)
    g1, be1 = f["g1"].astype(np.float64), f["be1"].astype(np.float64)
    g2, be2 = f["g2"].astype(np.float64), f["be2"].astype(np.float64)

    # grid / hat coefficients
    rg = np.linspace(0.0, RMAX, GRID_N)
    c = rg ** 2
    cl = np.empty(GRID_N); cr = np.empty(GRID_N)
    cl[1:] = c[:-1]; cl[0] = c[0] - 1.0
    cr[:-1] = c[1:]; cr[-1] = c[-1] + 1.0
    sl = 1.0 / (c - cl)
    sr = 1.0 / (cr - c)
    slq = np.asarray(sl, BF).astype(np.float64)     # quantized, used in PE
    coef = np.zeros((GRID_N, 3))
    coef[:, 0] = -cl * slq                           # A = psum + coef0
    coef[:, 1] = -sr / slq                           # B = psum*coef1+coef2
    coef[:, 2] = sr * cr

    ftab = np.zeros((GRID_N, NL * HID))
    for l in range(NL):
        ftab[:, l * HID:(l + 1) * HID] = _ssp_chain(rg, rw1[l], rw2[l], None)

    wgh = np.zeros((MUL, NL * MUL * HID))
    gfac = Y0 / math.sqrt(HID)
    for l in range(NL):
        r3 = rw3[l].reshape(HID, MUL, MUL)           # [h, i, j]
        wgl = gfac * r3.transpose(2, 1, 0)           # [j, i, h]
        wgl = wgl.reshape(MUL, MUL * HID)
        wgh[:, l * MUL * HID:(l + 1) * MUL * HID] = wgl

    norms = (geometry ** 2).sum(axis=-1)             # [B, N]
    f0_all = emb[features[..., 0]]                   # [B, N, EMB]

    w2c = np.zeros((128, MID))
    for k in range(2):
        w2c[:, k * 128:(k + 1) * 128] = W2[k * 128:(k + 1) * 128, :]

    msum = mask.sum(axis=1)                          # [B]
    shared = {
        "slrow": np.asarray(sl, BF).reshape(1, GRID_N),
        "coef": coef.astype(f32),
        "ftab": np.asarray(ftab, BF),
        "wg": np.asarray(wgh, BF),
        "mhead": np.repeat(mask.reshape(1, B * N), MUL, axis=0).astype(f32),
        "w1c": np.asarray(W1, BF),
        "b1c": b1.reshape(2, 128).T.astype(f32),
        "w2c": np.asarray(w2c, BF),
        "b2c": b2.reshape(128, 1).astype(f32),
        "wbar1": np.asarray(W1.sum(axis=1).reshape(EMB, 1), BF),
        "wbar2": np.asarray(
            np.stack([W2[k * 128:(k + 1) * 128, :].sum(axis=1)
                      for k in range(2)], axis=1), BF),
        "bbar": np.array([[b1.sum() / MID, b2.sum() / 128.0]], dtype=f32),
        "gb1": np.concatenate([g1, be1]).reshape(1, 2 * N).astype(f32),
        "gb2": np.concatenate([g2, be2]).reshape(1, 2 * N).astype(f32),
        "ones128": np.ones((128, 1), BF),
        "ones1": np.ones((1, 128), BF),
        "maskB": np.repeat(mask.reshape(1, B * N), 128, axis=0).astype(BF),
        "mvB": np.repeat((1.0 / msum).reshape(1, B), 128, axis=0).astype(f32),
        "epsv": np.full((1, 1), 1e-5, f32),
    }

    in_maps = []
    for core in range(NCORES):
        b = core // 2
        y0 = (core % 2) * YH
        geoYm = np.zeros((5, YH))
        geoYm[0:3] = -2.0 * geometry[b, y0:y0 + YH].T
        geoYm[3] = norms[b, y0:y0 + YH]
        geoYm[4] = 1.0
        geoXm = np.zeros((5, N))
        geoXm[0:3] = geometry[b].T
        geoXm[3] = 1.0
        geoXm[4] = norms[b]
        fm0 = (f0_all[b] * mask[b][:, None] * SQN).T    # [32, N]
        m = dict(shared)
        m["geoY"] = geoYm.astype(f32)
        m["geoX"] = geoXm.astype(f32)
        m["fm0"] = np.asarray(fm0[:, y0:y0 + YH], BF)
        m["msqn"] = np.repeat(
            (mask[b, y0:y0 + YH] * SQN).reshape(YH, 1), MUL, axis=1
        ).astype(f32)
        m["ident64"] = np.eye(YH, dtype=f32)
        in_maps.append(m)
    return in_maps


def run(inputs, trace=False):
    global _cached
    from concourse import bass_utils
    if _cached is None:
        _cached = _build()
    nc = _cached
    in_maps = _host_prep(inputs)
    res = bass_utils.run_bass_kernel_spmd(
        nc, in_maps, core_ids=list(range(NCORES)), trace=trace)
    return res


def kernel(**inputs):
    res = run(inputs, trace=False)
    return np.asarray(res.results[0]["out"], dtype=np.float32)


# revision 19
# speedup vs baseline: 1.1559x; 1.1559x over previous
"""Bass/Tile TRN2 kernel for nn_Network_21131239096982 (gnn_message_passing).

Sharding: 8 cores = 4 samples x 2 y-halves (full x per core). Pair order
(y outer, x inner). Per-layer ReduceScatter(cc_dim=Free) over the pair
sums the y-half partial preactivations and hands each core exactly its
own y-half of the next layer's features (rank0 -> cols 0:64, rank1 ->
cols 64:128), so the SPMD program needs no per-core offsets. Layer 3
does an 8-way AllGather of the [32, 128] partials; each core sums the
pair blocks and runs the batchnorm MLP head redundantly.

Key restructure vs the reference: the radial MLP (cosine basis -> ssp
-> ssp -> w3) is 128 univariate functions of r, host-tabulated on a
128-point r-uniform grid and evaluated on device as ONE matmul per pair
chunk:  s2[h, pair] = Ftab_l[g, h]^T @ O[g, pair],  where O holds
linear-interpolation hat weights in u = r^2 (asymmetric triangular
hats -> no sqrt needed). O is built once from a rank-1 PE broadcast
psum = sl_g * u plus three DVE ops per chunk. The gate
softplus(5t)/5 = relu(t) + poly9(min(|t|,2)) runs on DVE (+ Abs/Relu,
present in every ACT table set), so the only ACT table load in the
whole kernel is ln/exp for the two batchnorm inverse-stddevs.
"""

import math

import numpy as np

B, N, EMB, MUL = 4, 128, 32, 32
NB, MAXR = 10, 10.0
HID, BETA = 128, 5.0
MID, OUT = 256, 128
NL = 4
Y0 = 1.0 / (2.0 * math.sqrt(math.pi))
YH = N // 2          # 64 local y's per core
NP = N * YH          # 8192 pairs per core, order (y outer, x inner)
NCORES = 8
GRID_N = 128
RMAX = 7.5
SQN = 1.0 / math.sqrt(N)

# softplus(5t)/5 - relu(t) = ln(1+exp(-5|t|))/5, chebyshev fit on [0, 2]
GATE_PC = [0.13863592819866152, -0.4999284878393997, 0.6156649023363564,
           0.12337920499527943, -1.3216523110767724, 1.8311453040108088,
           -1.3266478452560657, 0.5557922376483523, -0.1274729154222193,
           0.012418893315223408]

_cached = None


def _build():
    import jax

    jax.devices()  # axon boot
    from concourse import bacc, tile, mybir

    F32 = mybir.dt.float32
    BF16 = mybir.dt.bfloat16
    AF = mybir.ActivationFunctionType
    ALU = mybir.AluOpType

    nc = bacc.Bacc("TRN2", debug=False, num_devices=NCORES)

    def din(name, shape, dt=F32):
        return nc.dram_tensor(name, shape, dt, kind="ExternalInput").ap()

    geoY_d = din("geoY", [5, YH])
    geoX_d = din("geoX", [5, N])
    slrow_d = din("slrow", [1, GRID_N], BF16)
    coef_d = din("coef", [GRID_N, 3])          # A-add, B-mult, B-add
    ftab_d = din("ftab", [GRID_N, NL * HID], BF16)
    wg_d = din("wg", [MUL, NL * MUL * HID], BF16)
    fm0_d = din("fm0", [MUL, YH], BF16)        # my y-half of fm layer 0
    msqn_d = din("msqn", [MUL, YH])            # ones32 x mask_half/sqrt(N)
    ident64_d = din("ident64", [YH, YH])
    mhead_d = din("mhead", [MUL, B * N])
    w1c_d = din("w1c", [EMB, MID], BF16)
    b1c_d = din("b1c", [128, 2])
    w2c_d = din("w2c", [128, MID], BF16)
    b2c_d = din("b2c", [128, 1])
    wbar1_d = din("wbar1", [EMB, 1], BF16)
    wbar2_d = din("wbar2", [128, 2], BF16)
    bbar_d = din("bbar", [1, 2])
    gb1_d = din("gb1", [1, 2 * N])             # g1r | be1r
    gb2_d = din("gb2", [1, 2 * N])             # g2r | be2r
    ones128_d = din("ones128", [128, 1], BF16)
    ones1_d = din("ones1", [1, 128], BF16)
    maskB_d = din("maskB", [128, B * N], BF16)
    mvB_d = din("mvB", [128, B])
    epsv_d = din("epsv", [1, 1])
    out_d = nc.dram_tensor("out", [B, OUT], F32, kind="ExternalOutput").ap()

    UMAX = (RMAX ** 2) * (1.0 - 1e-4)
    CH = 1024            # pair columns per psum tile (2 matmuls of 512)
    NCH = NP // CH       # 8

    with tile.TileContext(nc) as tc:
        with (
            tc.tile_pool(name="const", bufs=1) as cp,
            tc.tile_pool(name="s2p", bufs=3) as s2p,
            tc.tile_pool(name="gbp", bufs=2) as gbp,
            tc.tile_pool(name="fmp", bufs=2) as fmp,
            tc.tile_pool(name="wk", bufs=2) as wk,
            tc.tile_pool(name="hd", bufs=2) as hd,
            tc.tile_pool(name="ps_big", bufs=2, space="PSUM") as pA,
            tc.tile_pool(name="ps_g", bufs=2, space="PSUM") as pG,
            tc.tile_pool(name="ps_fc", bufs=2, space="PSUM") as pF,
            tc.tile_pool(name="dram", bufs=1, space="DRAM") as dp,
        ):
            def cload(ap, shape, dt=F32):
                t = cp.tile(shape, dt, name=ap.tensor.name + "_sb")
                nc.sync.dma_start(t[:], ap[:])
                return t

            geoY = cload(geoY_d, [5, YH])
            geoX = cload(geoX_d, [5, N])
            slrow = cload(slrow_d, [1, GRID_N], BF16)
            coef = cload(coef_d, [GRID_N, 3])
            fm0 = cload(fm0_d, [MUL, YH], BF16)
            msqn = cload(msqn_d, [MUL, YH])
            ident64 = cload(ident64_d, [YH, YH])
            ftab = cload(ftab_d, [GRID_N, NL * HID], BF16)
            mhead = cload(mhead_d, [MUL, B * N])
            w1c = cload(w1c_d, [EMB, MID], BF16)
            b1c = cload(b1c_d, [128, 2])
            w2c = cload(w2c_d, [128, MID], BF16)
            b2c = cload(b2c_d, [128, 1])
            wbar1 = cload(wbar1_d, [EMB, 1], BF16)
            wbar2 = cload(wbar2_d, [128, 2], BF16)
            bbar = cload(bbar_d, [1, 2])
            gb1 = cload(gb1_d, [1, 2 * N])
            gb2 = cload(gb2_d, [1, 2 * N])
            ones128 = cload(ones128_d, [128, 1], BF16)
            ones1 = cload(ones1_d, [1, 128], BF16)
            maskB = cload(maskB_d, [128, B * N], BF16)
            mvB = cload(mvB_d, [128, B])
            epsv = cload(epsv_d, [1, 1])

            # ---- act-table warmup (ln/exp set used by the BN head) ----
            tblw = wk.tile([1, 1], F32, name="tblw", tag="tblw", bufs=1)
            nc.scalar.activation(tblw[:], epsv[:], AF.Ln, bias=1.0)

            # ---- u = r^2 [y, x] clamped bf16; flatten via DRAM bounce ----
            r2ps = pG.tile([YH, N], F32, name="r2ps", tag="g")
            nc.tensor.matmul(r2ps[:], geoY[:], geoX[:], start=True, stop=True)
            u2d = wk.tile([YH, N], BF16, name="u2d", tag="u2d")
            nc.vector.tensor_scalar(
                u2d[:], r2ps[:], 0.0, UMAX, op0=ALU.max, op1=ALU.min)
            ubounce = dp.tile([YH, N], BF16, name="ubounce")
            nc.sync.dma_start(ubounce[:], u2d[:])
            urow = cp.tile([1, NP], BF16, name="urow")
            nc.sync.dma_start(
                urow[:], ubounce.opt().rearrange("p x -> () (p x)"))

            # ---- O[g, pair]: linear-interp hats in u ----
            # psum = sl_g*u ; A = psum + coef0 ; B = psum*coef1 + coef2
            # O = relu(min(A, B))
            obuf = cp.tile([GRID_N, NP], BF16, name="obuf")
            for c in range(NCH):
                ups = pA.tile([GRID_N, CH], F32, name="ups", tag="big")
                for h in range(2):
                    nc.tensor.matmul(
                        ups[:, h * 512:(h + 1) * 512], slrow[:],
                        urow[:, c * CH + h * 512:c * CH + (h + 1) * 512],
                        start=True, stop=True)
                osl = obuf[:, c * CH:(c + 1) * CH]
                bt = wk.tile([GRID_N, CH], F32, name="btile", tag="btile")
                nc.scalar.activation(bt[:], ups[:], AF.Identity,
                                     scale=coef[:, 1:2], bias=coef[:, 2:3])
                nc.vector.scalar_tensor_tensor(
                    osl, ups[:], coef[:, 0:1], bt[:],
                    op0=ALU.add, op1=ALU.min)
                nc.vector.tensor_scalar(osl, osl, 0.0, None, op0=ALU.max)

            # ---- gate helper ----
            def gate_chain(src_ap, pdim, width, mask_ap, name, res_dt=BF16):
                tt = wk.tile([pdim, width], F32, name=f"tt{name}", tag="gt", bufs=6)
                nc.scalar.activation(tt[:], src_ap, AF.Abs)
                nc.vector.tensor_scalar(tt[:], tt[:], 2.0, None, op0=ALU.min)
                # recurrence q=(q+c)*u gives a9*u^9+(c1)u^8+...+(c8)u,
                # so feed c_j = a_{9-j}; a0 folds into the final mask STT.
                pv = wk.tile([pdim, width], F32, name=f"pv{name}", tag="gt", bufs=6)
                nc.vector.tensor_scalar(
                    pv[:], tt[:], GATE_PC[9], None, op0=ALU.mult)
                for k in range(8, 0, -1):
                    nc.vector.scalar_tensor_tensor(
                        pv[:], pv[:], GATE_PC[k], tt[:],
                        op0=ALU.add, op1=ALU.mult)
                rl = wk.tile([pdim, width], F32, name=f"rl{name}", tag="gt", bufs=6)
                nc.scalar.activation(rl[:], src_ap, AF.Relu)
                nc.vector.tensor_tensor(pv[:], pv[:], rl[:], op=ALU.add)
                res = fmp.tile([pdim, width], res_dt, name=f"fm{name}", tag="fm")
                nc.vector.scalar_tensor_tensor(
                    res[:], pv[:], GATE_PC[0], mask_ap,
                    op0=ALU.add, op1=ALU.mult)
                return res

            # ---- conv layers ----
            fm = [fm0] + [None] * NL
            part3 = None

            def load_wg(l):
                t = gbp.tile([MUL, MUL * HID], BF16, name=f"wg{l}", tag="wg", bufs=4)
                nc.sync.dma_start(
                    t[:], wg_d[:, l * MUL * HID:(l + 1) * MUL * HID])
                return t

            def radial(l):
                s2 = s2p.tile([HID, NP], BF16, name=f"s2_{l}", tag="s2")
                for c in range(NCH):
                    rps = pA.tile([HID, CH], F32, name="rps", tag="big")
                    for h in range(2):
                        nc.tensor.matmul(
                            rps[:, h * 512:(h + 1) * 512],
                            ftab[:, l * HID:(l + 1) * HID],
                            obuf[:, c * CH + h * 512:c * CH + (h + 1) * 512],
                            start=True, stop=True)
                    dst = s2[:, c * CH:(c + 1) * CH]
                    if c % 2 == 0:
                        nc.scalar.activation(dst, rps[:], AF.Copy)
                    else:
                        nc.vector.tensor_copy(dst, rps[:])
                return s2

            wgl = [load_wg(l) for l in range(NL)]
            s2s = [None] * NL
            s2s[0] = radial(0)
            s2s[1] = radial(1)
            for l in range(NL):
                s2 = s2s[l]
                wg = wgl[l]
                # G-stage: gbuf[h, (i, y)] in blocks of 4 i's
                gbuf = gbp.tile([HID, MUL * YH], BF16, name=f"gb{l}", tag="gb")
                for q in range(MUL // 4):
                    gps = pG.tile([HID, 4 * YH], F32, name="gps", tag="g")
                    for k in range(4):
                        i = q * 4 + k
                        nc.tensor.matmul(
                            gps[:, k * YH:(k + 1) * YH],
                            wg[:, i * HID:(i + 1) * HID],
                            fm[l][:], start=True, stop=True)
                    nc.vector.tensor_copy(
                        gbuf[:, q * 4 * YH:(q + 1) * 4 * YH], gps[:])

                # final contraction over my y-half
                gview = gbuf[:].rearrange("p (i y) -> p y i", y=YH)
                if l < NL - 1:
                    # transposed partial [x, i]: flat RS halves == y-halves
                    pf = pF.tile([N, MUL], F32, name=f"pf{l}", tag="fc")
                    for y in range(YH):
                        nc.tensor.matmul(
                            pf[:], s2[:, y * N:(y + 1) * N], gview[:, y, :],
                            start=(y == 0), stop=(y == YH - 1))
                    part = wk.tile([N, MUL], F32, name=f"part{l}", tag="part")
                    nc.vector.tensor_copy(part[:], pf[:])
                    ari = dp.tile([N, MUL], F32, name=f"ari{l}")
                    nc.sync.dma_start(ari[:], part[:])
                    aro = dp.tile([YH, MUL], F32, name=f"aro{l}")
                    nc.gpsimd.collective_compute(
                        "ReduceScatter", ALU.add,
                        replica_groups=[[0, 1], [2, 3], [4, 5], [6, 7]],
                        ins=[ari.opt()], outs=[aro.opt()], cc_dim="Free")
                    pre = wk.tile([YH, MUL], F32, name=f"pre{l}", tag="pre")
                    nc.sync.dma_start(pre[:], aro.opt())
                    # transpose first: [y, i] -> psum [i, y]; gate reads psum
                    ftp = pG.tile([MUL, YH], F32, name=f"ftp{l}", tag="g")
                    nc.tensor.transpose(ftp[:], pre[:], ident64[:])
                    fmn = gate_chain(ftp[:], MUL, YH, msqn[:], f"{l}")
                    fm[l + 1] = fmn
                    if l + 2 < NL:
                        s2s[l + 2] = radial(l + 2)
                else:
                    pf = pF.tile([MUL, N], F32, name=f"pf{l}", tag="fc")
                    for y in range(YH):
                        nc.tensor.matmul(
                            pf[:], gview[:, y, :], s2[:, y * N:(y + 1) * N],
                            start=(y == 0), stop=(y == YH - 1))
                    part = wk.tile([MUL, N], F32, name=f"part{l}", tag="part")
                    nc.vector.tensor_copy(part[:], pf[:])
                    part3 = part

            # ---- layer-3 combine: 8-way AllGather, sum pair halves ----
            ag3i = dp.tile([MUL, N], F32, name="ag3i")
            nc.sync.dma_start(ag3i[:], part3[:])
            ag3o = dp.tile([NCORES * MUL, N], F32, name="ag3o")
            nc.gpsimd.collective_compute(
                "AllGather", ALU.bypass,
                replica_groups=[list(range(NCORES))],
                ins=[ag3i.opt()], outs=[ag3o.opt()])
            agv = ag3o.opt().rearrange("(b h i) x -> h i b x", h=2, i=MUL)
            t3a = hd.tile([MUL, B * N], F32, name="t3a", tag="t3")
            t3b = hd.tile([MUL, B * N], F32, name="t3b", tag="t3")
            nc.sync.dma_start(
                t3a[:].rearrange("i (b x) -> i b x", b=B), agv[0])
            nc.sync.dma_start(
                t3b[:].rearrange("i (b x) -> i b x", b=B), agv[1])
            nc.vector.tensor_tensor(t3a[:], t3a[:], t3b[:], op=ALU.add)
            fT = gate_chain(t3a[:], MUL, B * N, mhead[:], "hd")

            # ---- head: 2x (linear + BN + lrelu), masked mean pool ----
            BN_ = B * N

            def bn_block(rhs_tiles, wts, wbars, bbar_ap, bias, gbrow, cnt,
                         nchunk):
                """rhs_tiles: bf16 [K, BN_] chunks; wts[m][kk]: lhsT APs;
                wbars[kk]: [K, 1] col-sum lhsT APs."""
                mups = pG.tile([1, BN_], F32, name="mups", tag="g")
                for kk, rt in enumerate(rhs_tiles):
                    nc.tensor.matmul(
                        mups[:], wbars[kk], rt[:],
                        start=(kk == 0), stop=(kk == len(rhs_tiles) - 1))
                mu = hd.tile([1, N], F32, name="mu", tag="r128", bufs=10)
                nc.vector.reduce_sum(
                    mu[:], mups[:].rearrange("p (b x) -> p x b", b=B),
                    axis=mybir.AxisListType.X)
                nc.vector.tensor_scalar(
                    mu[:], mu[:], 1.0 / cnt, bbar_ap, op0=ALU.mult, op1=ALU.add)
                asb, sqs = [], []
                for m in range(nchunk):
                    aps = pA.tile([128, BN_], F32, name=f"aps{m}", tag="big")
                    for kk, rt in enumerate(rhs_tiles):
                        nc.tensor.matmul(
                            aps[:], wts[m][kk], rt[:],
                            start=(kk == 0),
                            stop=(kk == len(rhs_tiles) - 1))
                    av = hd.tile([128, BN_], F32, name=f"av{m}", tag="av", bufs=3)
                    nc.vector.tensor_scalar(
                        av[:], aps[:], bias[:, m:m + 1], None, op0=ALU.add)
                    sq = hd.tile([128, BN_], BF16, name=f"sq{m}", tag="sq", bufs=3)
                    nc.scalar.activation(
                        sq[:], aps[:], AF.Square, bias=bias[:, m:m + 1])
                    asb.append(av)
                    sqs.append(sq)
                qps = pG.tile([1, BN_], F32, name="qps", tag="g")
                for m in range(nchunk):
                    nc.tensor.matmul(qps[:], ones128[:], sqs[m][:],
                                     start=(m == 0), stop=(m == nchunk - 1))
                var = hd.tile([1, N], F32, name="var", tag="r128", bufs=10)
                nc.vector.reduce_sum(
                    var[:], qps[:].rearrange("p (b x) -> p x b", b=B),
                    axis=mybir.AxisListType.X)
                nc.vector.tensor_scalar_mul(var[:], var[:], 1.0 / cnt)
                musq = hd.tile([1, N], F32, name="musq", tag="r128", bufs=10)
                nc.vector.tensor_tensor(musq[:], mu[:], mu[:], op=ALU.mult)
                nc.vector.tensor_tensor(
                    var[:], var[:], musq[:], op=ALU.subtract)
                inv = hd.tile([1, N], F32, name="inv", tag="r128", bufs=10)
                nc.scalar.activation(inv[:], var[:], AF.Ln, bias=epsv[:, 0:1])
                nc.scalar.activation(inv[:], inv[:], AF.Exp, scale=-0.5)
                svec = hd.tile([1, N], F32, name="svec", tag="r128", bufs=10)
                nc.vector.tensor_tensor(
                    svec[:], gbrow[:, 0:N], inv[:], op=ALU.mult)
                tvec = hd.tile([1, N], F32, name="tvec", tag="r128", bufs=10)
                nc.vector.tensor_tensor(tvec[:], mu[:], svec[:], op=ALU.mult)
                nc.vector.scalar_tensor_tensor(
                    tvec[:], tvec[:], -1.0, gbrow[:, N:2 * N],
                    op0=ALU.mult, op1=ALU.add)
                sw = hd.tile([1, BN_], BF16, name="sw", tag="row", bufs=6)
                tw = hd.tile([1, BN_], BF16, name="tw", tag="row", bufs=6)
                for b in range(B):
                    nc.vector.tensor_copy(sw[:, b * N:(b + 1) * N], svec[:])
                    nc.vector.tensor_copy(tw[:, b * N:(b + 1) * N], tvec[:])
                sB = pG.tile([128, BN_], F32, name="sB", tag="g")
                tB = pG.tile([128, BN_], F32, name="tB", tag="g")
                nc.tensor.matmul(sB[:], ones1[:], sw[:], start=True, stop=True)
                nc.tensor.matmul(tB[:], ones1[:], tw[:], start=True, stop=True)
                outs = []
                for m in range(nchunk):
                    nc.vector.tensor_tensor(
                        asb[m][:], asb[m][:], sB[:], op=ALU.mult)
                    nc.vector.tensor_tensor(
                        asb[m][:], asb[m][:], tB[:], op=ALU.add)
                    ho = hd.tile([128, BN_], BF16, name=f"ho{m}", tag="ho", bufs=4)
                    nc.scalar.activation(ho[:], asb[m][:], AF.Prelu, alpha=0.2)
                    outs.append(ho)
                return outs

            w1chunks = [[w1c[:, m * 128:(m + 1) * 128]] for m in range(2)]
            h1 = bn_block([fT], w1chunks, [wbar1[:]], bbar[:, 0:1],
                          b1c, gb1, float(B * MID), 2)
            w2chunks = [[w2c[:, k * 128:(k + 1) * 128] for k in range(2)]]
            h2 = bn_block(h1, w2chunks,
                          [wbar2[:, 0:1], wbar2[:, 1:2]], bbar[:, 1:2],
                          b2c, gb2, float(B * 128), 1)

            # masked mean pool -> out[b, o]
            h2m = hd.tile([128, BN_], BF16, name="h2m", tag="ho", bufs=4)
            nc.vector.tensor_tensor(h2m[:], h2[0][:], maskB[:], op=ALU.mult)
            pool = hd.tile([128, B], F32, name="pool", tag="pool")
            nc.vector.reduce_sum(
                pool[:], h2m[:].rearrange("p (b x) -> p b x", b=B),
                axis=mybir.AxisListType.X)
            nc.vector.tensor_tensor(pool[:], pool[:], mvB[:], op=ALU.mult)
            nc.sync.dma_start(out_d[:].rearrange("b o -> o b"), pool[:])

    nc.compile()
    return nc


def _ssp_chain(r, rw1, rw2, rw3_unused):
    grid = np.linspace(0.0, MAXR, NB)
    step = grid[1] - grid[0]
    x = (r[..., None] - grid) / step
    basis = np.where(np.abs(x) < 1.0, np.cos(0.5 * math.pi * x) ** 2, 0.0)

    def ssp(v):
        return (np.logaddexp(0, BETA * v) - math.log(2.0)) / BETA

    h = ssp(basis @ rw1 / math.sqrt(NB))
    h = ssp(h @ rw2 / math.sqrt(HID))
    return h


def _host_prep(inputs):
    import ml_dtypes
    BF = ml_dtypes.bfloat16
    f32 = np.float32

    f = {k: np.asarray(v) for k, v in inputs.items()}
    geometry = f["geometry"].astype(np.float64)
    features = f["features"].astype(np.int64)
    mask = f["mask"].astype(np.float64)
    emb = f["emb"].astype(np.float64)
    rw1, rw2, rw3 = (f[k].astype(np.float64) for k in ("rw1", "rw2", "rw3"))
    W1, b1 = f["W1"].astype(np.float64), f["b1"].astype(np.float64)
    W2, b2 = f["W2"].astype(np.float64), f["b2"].astype(np.float64)
    g1, be1 = f["g1"].astype(np.float64), f["be1"].astype(np.float64)
    g2, be2 = f["g2"].astype(np.float64), f["be2"].astype(np.float64)

    # grid / hat coefficients
    rg = np.linspace(0.0, RMAX, GRID_N)
    c = rg ** 2
    cl = np.empty(GRID_N); cr = np.empty(GRID_N)
    cl[1:] = c[:-1]; cl[0] = c[0] - 1.0
    cr[:-1] = c[1:]; cr[-1] = c[-1] + 1.0
    sl = 1.0 / (c - cl)
    sr = 1.0 / (cr - c)
    slq = np.asarray(sl, BF).astype(np.float64)     # quantized, used in PE
    coef = np.zeros((GRID_N, 3))
    coef[:, 0] = -cl * slq                           # A = psum + coef0
    coef[:, 1] = -sr / slq                           # B = psum*coef1+coef2
    coef[:, 2] = sr * cr

    ftab = np.zeros((GRID_N, NL * HID))
    for l in range(NL):
        ftab[:, l * HID:(l + 1) * HID] = _ssp_chain(rg, rw1[l], rw2[l], None)

    wgh = np.zeros((MUL, NL * MUL * HID))
    gfac = Y0 / math.sqrt(HID)
    for l in range(NL):
        r3 = rw3[l].reshape(HID, MUL, MUL)           # [h, i, j]
        wgl = gfac * r3.transpose(2, 1, 0)           # [j, i, h]
        wgl = wgl.reshape(MUL, MUL * HID)
        wgh[:, l * MUL * HID:(l + 1) * MUL * HID] = wgl

    norms = (geometry ** 2).sum(axis=-1)             # [B, N]
    f0_all = emb[features[..., 0]]                   # [B, N, EMB]

    w2c = np.zeros((128, MID))
    for k in range(2):
        w2c[:, k * 128:(k + 1) * 128] = W2[k * 128:(k + 1) * 128, :]

    msum = mask.sum(axis=1)                          # [B]
    shared = {
        "slrow": np.asarray(sl, BF).reshape(1, GRID_N),
        "coef": coef.astype(f32),
        "ftab": np.asarray(ftab, BF),
        "wg": np.asarray(wgh, BF),
        "mhead": np.repeat(mask.reshape(1, B * N), MUL, axis=0).astype(f32),
        "w1c": np.asarray(W1, BF),
        "b1c": b1.reshape(2, 128).T.astype(f32),
        "w2c": np.asarray(w2c, BF),
        "b2c": b2.reshape(128, 1).astype(f32),
        "wbar1": np.asarray(W1.sum(axis=1).reshape(EMB, 1), BF),
        "wbar2": np.asarray(
            np.stack([W2[k * 128:(k + 1) * 128, :].sum(axis=1)
                      for k in range(2)], axis=1), BF),
        "bbar": np.array([[b1.sum() / MID, b2.sum() / 128.0]], dtype=f32),
        "gb1": np.concatenate([g1, be1]).reshape(1, 2 * N).astype(f32),
        "gb2": np.concatenate([g2, be2]).reshape(1, 2 * N).astype(f32),
        "ones128": np.ones((128, 1), BF),
        "ones1": np.ones((1, 128), BF),
        "maskB": np.repeat(mask.reshape(1, B * N), 128, axis=0).astype(BF),
        "mvB": np.repeat((1.0 / msum).reshape(1, B), 128, axis=0).astype(f32),
        "epsv": np.full((1, 1), 1e-5, f32),
    }

    in_maps = []
    for core in range(NCORES):
        b = core // 2
        y0 = (core % 2) * YH
        geoYm = np.zeros((5, YH))
        geoYm[0:3] = -2.0 * geometry[b, y0:y0 + YH].T
        geoYm[3] = norms[b, y0:y0 + YH]
        geoYm[4] = 1.0
        geoXm = np.zeros((5, N))
        geoXm[0:3] = geometry[b].T
        geoXm[3] = 1.0
        geoXm[4] = norms[b]
        fm0 = (f0_all[b] * mask[b][:, None] * SQN).T    # [32, N]
        m = dict(shared)
        m["geoY"] = geoYm.astype(f32)
        m["geoX"] = geoXm.astype(f32)
        m["fm0"] = np.asarray(fm0[:, y0:y0 + YH], BF)
        m["msqn"] = np.repeat(
            (mask[b, y0:y0 + YH] * SQN).reshape(1, YH), MUL, axis=0
        ).astype(f32)
        m["ident64"] = np.eye(YH, dtype=f32)
        in_maps.append(m)
    return in_maps


def run(inputs, trace=False):
    global _cached
    from concourse import bass_utils
    if _cached is None:
        _cached = _build()
    nc = _cached
    in_maps = _host_prep(inputs)
    res = bass_utils.run_bass_kernel_spmd(
        nc, in_maps, core_ids=list(range(NCORES)), trace=trace)
    return res


def kernel(**inputs):
    res = run(inputs, trace=False)
    return np.asarray(res.results[0]["out"], dtype=np.float32)
